# revision 7
# baseline (speedup 1.0000x reference)
"""Contrastive (NT-Xent) loss kernel for TRN2, 8 NeuronCores.

Reference math: p = concat(proj_i, proj_j) [N=8192, D=128]; z = row-normalized p;
sim = z @ z.T; loss = -(1/N) sum_r [ 2*sim[r, partner(r)] - ln(sum_{c!=r} exp(2 sim[r,c])) ]
with partner(r) = (r+B) mod N. sim in [-1,1] so exp(2 sim) in [e^-2, e^2]: no
max-subtraction needed.

sim is SYMMETRIC: only half the exp work is needed. Circulant macro-block
decomposition: split N into 16 macro blocks of 512 rows. Core c computes
  L0: rows of macro c    x cols of macros {c..c+8 mod 16}    (9 groups)
  L8: rows of macro c+8  x cols of macros {c+8..c+15 mod 16} (8 groups)
With p ROLLED by 512*c rows per core, every core runs the identical program:
  L0 = local chunks 0..3   x local cols [0, 4608)
  L8 = local chunks 32..35 x local cols [4096, 8192)
Every global ordered pair (r,c) lands in exactly one computed tile (checked
combinatorially: difference d = macro(c)-macro(r) mod 16 in 0..8 via L0 rows,
d in 0..7 via L8 rows). Row sums of exp come from the ACT accum_out; the
mirrored direction's row sums are the COLUMN sums of the computed exp tiles,
obtained by ones-matmuls on the tensor engine (fp8 DoubleRow: two 128-row
chunks = K=256 per pass), accumulated in PSUM per 512-col group. The host
adds both parts, subtracts the self term e^2, and finishes log/mean in fp64.

Per core:
 1. p arrives as bf16 (host casts; kernel-internal precision is bf16 anyway).
    16 batches of 512 rows stream in; DVE computes sumsq (square+reduce) and
    rsqrt via constant-seed Newton (4 steps, pure mul/add -> NO activation
    table loads ever besides exp), scales to z bf16, stages to DRAM,
    xbar-transposes to zT.
 2. Main loop in 6 waves as zT groups land: per chunk a [128,1536] PSUM tile
    (3 matmuls of 512) -> one ACT Exp(scale=2, fp8 out) with fused accum_out
    row-sum. Partner diagonal read off the raw PSUM tile (exact f32).
 3. Colsums: per 512-col group, ones^T @ es via fp8 DoubleRow matmuls
    accumulating in a [1,512] PSUM slot; DVE spills to bf16; DMA out.
"""

import numpy as np

import concourse.bass as bass
import concourse.mybir as mybir
import concourse.tile as tile
from concourse import bacc
from concourse.bass_utils import run_bass_kernel_spmd
from concourse.masks import make_identity

B = 4096
D = 128
N = 2 * B
NCORES = 8
P = 128
NB = 16                  # 512-row batches / 512-col groups
E2 = float(np.exp(np.float64(2.0)))

# batch issue order: L0 needs groups 0..8, L8 needs 8..15; ordered so each
# wave's gating groups arrive earliest.
ISSUE = [0, 1, 2, 8, 9, 10, 3, 4, 5, 11, 12, 13, 6, 7, 14, 15]
TPOS = {g: t for t, g in enumerate(ISSUE)}

L0_SLOTS = [0, 1, 2, 3]          # local chunks 0..3   (rows 128*slot+p)
L8_SLOTS = [4, 5, 6, 7]          # local chunks 32..35 (rows 4096+128*(slot-4)+p)
L0_GSETS = [[0, 1, 2], [3, 4, 5], [6, 7, 8]]
L8_GSETS = [[8, 9, 10], [11, 12, 13], [14, 15]]
WAVES = sorted(
    [("L0", k, max(TPOS[g] for g in gs)) for k, gs in enumerate(L0_GSETS)]
    + [("L8", k, max(TPOS[g] for g in gs)) for k, gs in enumerate(L8_GSETS)],
    key=lambda w: w[2],
)
# colsum groups (diag groups 0 and 8 excluded) -> cols output row index
CS_GROUPS = list(range(1, 9)) + list(range(9, 16))   # 15 groups

f32 = mybir.dt.float32
bf16 = mybir.dt.bfloat16
fp8 = mybir.dt.float8e4
Act = mybir.ActivationFunctionType
Alu = mybir.AluOpType
AxX = mybir.AxisListType.X
DR = mybir.MatmulPerfMode.DoubleRow

C_SEED = float(1.0 / np.sqrt(128.0))   # Newton rsqrt seed; ssq ~ chi2_128


def _chunk_cols(slot):
    """zT (group, offset) of this slot's own columns."""
    if slot < 4:
        return 0, 128 * slot
    return 8, 128 * (slot - 4)


def _build_kernel(tc: tile.TileContext, part_ap: bass.AP, cols_ap: bass.AP,
                  pc_ap: bass.AP):
    nc = tc.nc
    with (
        tc.tile_pool(name="zt", bufs=1) as ztp,
        tc.tile_pool(name="io", bufs=16) as iop,
        tc.tile_pool(name="zo", bufs=3) as zop,
        tc.tile_pool(name="tmp", bufs=3) as tmp,
        tc.tile_pool(name="small", bufs=1) as smallp,
        tc.tile_pool(name="es", bufs=1) as esp,
        tc.tile_pool(name="sp", bufs=3) as spp,
        tc.tile_pool(name="ps", bufs=2, space="PSUM") as psp,
        tc.tile_pool(name="cs", bufs=1, space="PSUM") as csp,
        tc.tile_pool(name="zd", bufs=1, space="DRAM") as zdp,
    ):
        ident = smallp.tile([P, P], bf16, tag="ident")
        make_identity(nc, ident[:])
        ones2 = smallp.tile([P, 2, 64], fp8, tag="ones2")
        nc.gpsimd.memset(ones2[:], 1.0)

        zT = ztp.tile([P, NB, 512], bf16, tag="zT")        # [d, group, col]
        es = esp.tile([P, 8, 3, 1536], fp8, tag="es")      # [row, slot, k, col]
        ssq = smallp.tile([P, NB, 4], f32, tag="ssq")      # by issue position
        ya = smallp.tile([P, NB, 4], f32, tag="ya")
        yb = smallp.tile([P, NB, 4], f32, tag="yb")
        ta = smallp.tile([P, NB, 4], f32, tag="ta")
        tb = smallp.tile([P, NB, 4], f32, tag="tb")
        rn = smallp.tile([P, NB, 4], f32, tag="rn")
        sums = smallp.tile([P, 8, 3], f32, tag="sums")
        pos = smallp.tile([P, 4], f32, tag="pos")

        zdram = zdp.tile([N, D], bf16, tag="zd", name="zd")

        pts = {}

        def emit_batch_pair(pi):
            """Load+norm+stage+transpose for issue positions 2pi, 2pi+1."""
            t0, t1 = 2 * pi, 2 * pi + 1
            for t in (t0, t1):
                g = ISSUE[t]
                pt = iop.tile([P, 4, D], bf16, tag="pt", name=f"pt_{t}")
                nc.sync.dma_start(
                    pt[:],
                    pc_ap[512 * g:512 * (g + 1), :]
                    .rearrange("(u p) d -> p u d", p=P),
                )
                pts[t] = pt
                sq = tmp.tile([P, 4, D], bf16, tag="sq")
                nc.vector.tensor_mul(sq[:], pt[:], pt[:])
                nc.vector.reduce_sum(ssq[:, t, :], sq[:], axis=AxX)
            # Newton rsqrt on [128, 2, 4]: y1 = A - B*x (from const seed),
            # then 3 iterations y <- y*(1.5 - 0.5*x*y^2). No in-place ops.
            s = (slice(None), slice(t0, t1 + 1), slice(None))
            nc.vector.tensor_scalar(
                ya[s], ssq[s], -0.5 * C_SEED**3, 1.5 * C_SEED, Alu.mult, Alu.add
            )
            for it, (yi, yo) in enumerate(((ya, yb), (yb, ya), (ya, None))):
                nc.vector.tensor_mul(ta[s], yi[s], yi[s])
                nc.vector.tensor_mul(tb[s], ta[s], ssq[s])
                nc.vector.tensor_scalar(ta[s], tb[s], -0.5, 1.5, Alu.mult, Alu.add)
                if yo is not None:
                    nc.vector.tensor_mul(yo[s], yi[s], ta[s])
                else:
                    nc.vector.tensor_mul(rn[s], yi[s], ta[s])
            for t in (t0, t1):
                g = ISSUE[t]
                zt4 = zop.tile([P, 4, D], bf16, tag="zt4")
                for j in range(4):
                    nc.vector.tensor_scalar_mul(
                        zt4[:, j, :], pts[t][:, j, :], rn[:, t, j:j + 1]
                    )
                nc.sync.dma_start(
                    zdram[512 * g:512 * (g + 1), :]
                    .rearrange("(u p) d -> p u d", p=P),
                    zt4[:],
                )
                nc.sync.dma_start_transpose(
                    zT[:, g, :], zdram[512 * g:512 * (g + 1), :]
                )

        def emit_wave(block, k):
            slots = L0_SLOTS if block == "L0" else L8_SLOTS
            gset = (L0_GSETS if block == "L0" else L8_GSETS)[k]
            w = 512 * len(gset)
            for slot in slots:
                lg, lo = _chunk_cols(slot)
                lhsT = zT[:, lg, lo:lo + P]
                ps = psp.tile([P, 1536], f32, tag="ps")
                for i, g in enumerate(gset):
                    nc.tensor.matmul(
                        ps[:, 512 * i:512 * (i + 1)], lhsT, zT[:, g, :],
                        start=True, stop=True,
                    )
                # partner diagonal from raw PSUM (exact): L0 k2 has g8 at
                # tile cols [1024, 1536); partner of row 128m+p is col 128m+p
                # within that block.
                if block == "L0" and k == 2:
                    m = slot
                    sq2 = tmp.tile([P, P], f32, tag="sq2")
                    nc.vector.tensor_mul(
                        sq2[:], ps[:, 1024 + 128 * m:1152 + 128 * m], ident[:]
                    )
                    nc.vector.reduce_sum(pos[:, m:m + 1], sq2[:], axis=AxX)
                nc.scalar.activation(
                    es[:, slot, k, 0:w], ps[:, 0:w], Act.Exp, scale=2.0,
                    accum_out=sums[:, slot, k:k + 1],
                )
            # colsums for the groups just exp'd. Skip only this block's OWN
            # diag tile (L0@g0, L8@g8): its mirror pairs are computed
            # directly within the block. L0@g8 (macro pair (0,8), d=8) DOES
            # need a colsum - pair (8,0) is never computed directly.
            diag_g = 0 if block == "L0" else 8
            for i, g in enumerate(gset):
                if g == diag_g:
                    continue
                cs = csp.tile([64, 512], f32, tag="cs")
                for pj, s0 in enumerate((slots[0], slots[2])):
                    nc.tensor.matmul(
                        cs[:], ones2[:],
                        es[:, s0:s0 + 2, k, 512 * i:512 * (i + 1)],
                        start=(pj == 0), stop=(pj == 1), perf_mode=DR,
                    )
                sp = spp.tile([1, 512], bf16, tag="sp")
                nc.vector.tensor_copy(sp[:], cs[0:1, :])
                nc.sync.dma_start(cols_ap[CS_GROUPS.index(g), :], sp[:])

        wi = 0
        for pi in range(8):
            emit_batch_pair(pi)
            while wi < len(WAVES) and WAVES[wi][2] <= 2 * pi + 1:
                emit_wave(WAVES[wi][0], WAVES[wi][1])
                wi += 1
        assert wi == len(WAVES)

        nc.sync.dma_start(
            part_ap[:, 0:24], sums[:].rearrange("p a b -> p (a b)")
        )
        nc.sync.dma_start(part_ap[:, 24:28], pos[:])


_CACHE: dict = {}


def _compiled():
    if "nc" not in _CACHE:
        nc = bacc.Bacc(
            "TRN2", target_bir_lowering=False, debug=False,
            enable_asserts=True, num_devices=NCORES,
        )
        pc = nc.dram_tensor("pc", [N, D], bf16, kind="ExternalInput").ap()
        part = nc.dram_tensor("partial", [P, 28], f32, kind="ExternalOutput").ap()
        cols = nc.dram_tensor("cols", [15, 512], bf16, kind="ExternalOutput").ap()
        with tile.TileContext(nc) as tc:
            _build_kernel(tc, part, cols, pc)
        nc.compile()
        _CACHE["nc"] = nc
    return _CACHE["nc"]


def kernel(proj_i: np.ndarray, proj_j: np.ndarray, **run_kwargs) -> np.ndarray:
    import ml_dtypes

    assert proj_i.shape == (B, D) and proj_j.shape == (B, D)
    nc = _compiled()
    p = np.concatenate(
        [np.asarray(proj_i, np.float32), np.asarray(proj_j, np.float32)], axis=0
    ).astype(ml_dtypes.bfloat16)
    in_maps = [
        {"pc": np.ascontiguousarray(np.roll(p, -512 * c, axis=0))}
        for c in range(NCORES)
    ]
    res = run_bass_kernel_spmd(nc, in_maps, list(range(NCORES)), **run_kwargs)

    rowsum = np.zeros(N, np.float64)
    posg = np.zeros(N, np.float64)
    for c, r in enumerate(res.results):
        part = np.asarray(r["partial"], np.float64)      # [128, 28]
        cols = np.asarray(r["cols"], np.float64)         # [15, 512]
        sums = part[:, :24].reshape(P, 8, 3).sum(axis=2)  # [128, slot]
        for slot in range(8):
            base = 128 * slot if slot < 4 else 4096 + 128 * (slot - 4)
            rows = (512 * c + base + np.arange(P)) % N
            rowsum[rows] += sums[:, slot]
        for i, g in enumerate(CS_GROUPS):
            rows = (512 * c + 512 * g + np.arange(512)) % N
            rowsum[rows] += cols[i, :]
        for m in range(4):
            rows = (512 * c + 128 * m + np.arange(P)) % N
            posg[rows] = part[:, 24 + m]
            posg[(rows + B) % N] = part[:, 24 + m]
    _CACHE["last_results"] = res
    loss = -(2.0 * posg - np.log(rowsum - E2)).sum() / N
    return np.float32(loss)


# revision 8
# speedup vs baseline: 1.2676x; 1.2676x over previous
"""Contrastive (NT-Xent) loss kernel for TRN2, 8 NeuronCores.

Reference math: p = concat(proj_i, proj_j) [N=8192, D=128]; z = row-normalized p;
sim = z @ z.T; loss = -(1/N) sum_r [ 2*sim[r, partner(r)] - ln(sum_{c!=r} exp(2 sim[r,c])) ]
with partner(r) = (r+B) mod N. sim in [-1,1] so exp(2 sim) in [e^-2, e^2]: no
max-subtraction needed.

sim is SYMMETRIC: only half the exp work is needed. Circulant macro-block
decomposition: split N into 16 macro blocks of 512 rows. Core c computes
  L0: rows of macro c    x cols of macros {c..c+8 mod 16}    (9 groups)
  L8: rows of macro c+8  x cols of macros {c+8..c+15 mod 16} (8 groups)
With p ROLLED by 512*c rows per core, every core runs the identical program:
  L0 = local chunks 0..3   x local cols [0, 4608)
  L8 = local chunks 32..35 x local cols [4096, 8192)
Every global ordered pair (r,c) lands in exactly one computed tile (checked
combinatorially: difference d = macro(c)-macro(r) mod 16 in 0..8 via L0 rows,
d in 0..7 via L8 rows). Row sums of exp come from the ACT accum_out; the
mirrored direction's row sums are the COLUMN sums of the computed exp tiles,
obtained by ones-matmuls on the tensor engine (fp8 DoubleRow: two 128-row
chunks = K=256 per pass), accumulated in PSUM per 512-col group. The host
adds both parts, subtracts the self term e^2, and finishes log/mean in fp64.

Per core:
 1. p arrives as bf16 (host casts; kernel-internal precision is bf16 anyway).
    16 batches of 512 rows stream in; DVE computes sumsq (square+reduce) and
    rsqrt via constant-seed Newton (4 steps, pure mul/add -> NO activation
    table loads ever besides exp), scales to z bf16, stages to DRAM,
    xbar-transposes to zT.
 2. Main loop in 6 waves as zT groups land: per chunk a [128,1536] PSUM tile
    (3 matmuls of 512) -> one ACT Exp(scale=2, fp8 out) with fused accum_out
    row-sum. Partner diagonal read off the raw PSUM tile (exact f32).
 3. Colsums: per 512-col group, ones^T @ es via fp8 DoubleRow matmuls
    accumulating in a [1,512] PSUM slot; DVE spills to bf16; DMA out.
"""

import numpy as np

import concourse.bass as bass
import concourse.mybir as mybir
import concourse.tile as tile
from concourse import bacc
from concourse.bass_utils import run_bass_kernel_spmd
from concourse.masks import make_identity

B = 4096
D = 128
N = 2 * B
NCORES = 8
P = 128
NB = 16                  # 512-row batches / 512-col groups
E2 = float(np.exp(np.float64(2.0)))

# batch issue order: L0 needs groups 0..8, L8 needs 8..15; ordered so each
# wave's gating groups arrive earliest.
ISSUE = [0, 1, 2, 3, 8, 9, 4, 5, 10, 11, 6, 7, 12, 13, 14, 15]
TPOS = {g: t for t, g in enumerate(ISSUE)}

L0_SLOTS = [0, 1, 2, 3]          # local chunks 0..3   (rows 128*slot+p)
L8_SLOTS = [4, 5, 6, 7]          # local chunks 32..35 (rows 4096+128*(slot-4)+p)
L0_GSETS = [[0, 1, 2], [3, 4, 5], [6, 7, 8]]
L8_GSETS = [[8, 9, 10], [11, 12, 13], [14, 15]]
WAVES = sorted(
    [("L0", k, max(TPOS[g] for g in gs)) for k, gs in enumerate(L0_GSETS)]
    + [("L8", k, max(TPOS[g] for g in gs)) for k, gs in enumerate(L8_GSETS)],
    key=lambda w: w[2],
)
# colsum groups (diag groups 0 and 8 excluded) -> cols output row index
CS_GROUPS = list(range(1, 9)) + list(range(9, 16))   # 15 groups

f32 = mybir.dt.float32
bf16 = mybir.dt.bfloat16
fp8 = mybir.dt.float8e4
Act = mybir.ActivationFunctionType
Alu = mybir.AluOpType
AxX = mybir.AxisListType.X
DR = mybir.MatmulPerfMode.DoubleRow

C_SEED = float(1.0 / np.sqrt(128.0))   # Newton rsqrt seed; ssq ~ chi2_128


def _chunk_cols(slot):
    """zT (group, offset) of this slot's own columns."""
    if slot < 4:
        return 0, 128 * slot
    return 8, 128 * (slot - 4)


def _build_kernel(tc: tile.TileContext, part_ap: bass.AP, cols_ap: bass.AP,
                  pc_ap: bass.AP):
    nc = tc.nc
    with (
        tc.tile_pool(name="zt", bufs=1) as ztp,
        tc.tile_pool(name="io", bufs=8) as iop,
        tc.tile_pool(name="zo", bufs=3) as zop,
        tc.tile_pool(name="tmp", bufs=3) as tmp,
        tc.tile_pool(name="small", bufs=1) as smallp,
        tc.tile_pool(name="es", bufs=1) as esp,
        tc.tile_pool(name="sp", bufs=3) as spp,
        tc.tile_pool(name="ps", bufs=2, space="PSUM") as psp,
        tc.tile_pool(name="cs", bufs=1, space="PSUM") as csp,
        tc.tile_pool(name="zd", bufs=1, space="DRAM") as zdp,
    ):
        ident = smallp.tile([P, P], bf16, tag="ident")
        make_identity(nc, ident[:])
        ones2 = smallp.tile([P, 2, 64], fp8, tag="ones2")
        nc.gpsimd.memset(ones2[:], 1.0)

        zT = ztp.tile([P, NB, 512], bf16, tag="zT")        # [d, group, col]
        es = esp.tile([P, 8, 3, 1536], fp8, tag="es")      # [row, slot, k, col]
        ssq = smallp.tile([P, NB, 4], f32, tag="ssq")      # by issue position
        ya = smallp.tile([P, NB, 4], f32, tag="ya")
        yb = smallp.tile([P, NB, 4], f32, tag="yb")
        ta = smallp.tile([P, NB, 4], f32, tag="ta")
        tb = smallp.tile([P, NB, 4], f32, tag="tb")
        rn = smallp.tile([P, NB, 4], f32, tag="rn")
        sums = smallp.tile([P, 8, 3], f32, tag="sums")
        pos = smallp.tile([P, 4], f32, tag="pos")

        zdram = zdp.tile([N, D], bf16, tag="zd", name="zd")

        pts = {}

        def emit_in(pi):
            """Paired input DMA for issue positions 2pi, 2pi+1 (adjacent
            groups g, g+1): issued up front with no waits."""
            g = ISSUE[2 * pi]
            assert ISSUE[2 * pi + 1] == g + 1
            pt = iop.tile([P, 8, D], bf16, tag="pt", name=f"pt_{pi}")
            nc.sync.dma_start(
                pt[:],
                pc_ap[512 * g:512 * g + 1024, :]
                .rearrange("(u p) d -> p u d", p=P),
            )
            pts[pi] = pt

        def emit_batch_pair(pi):
            """Norm+stage+transpose for issue positions 2pi, 2pi+1."""
            t0 = 2 * pi
            g = ISSUE[t0]
            pt = pts[pi]
            sq = tmp.tile([P, 8, D], bf16, tag="sq")
            nc.vector.tensor_mul(sq[:], pt[:], pt[:])
            nc.vector.reduce_sum(
                ssq[:, t0:t0 + 2, :].rearrange("p a b -> p (a b)"), sq[:],
                axis=AxX,
            )
            # Newton rsqrt on [128, 2, 4]: y1 = A - B*x (from const seed),
            # then 3 iterations y <- y*(1.5 - 0.5*x*y^2). No in-place ops.
            s = (slice(None), slice(t0, t0 + 2), slice(None))
            nc.vector.tensor_scalar(
                ya[s], ssq[s], -0.5 * C_SEED**3, 1.5 * C_SEED, Alu.mult, Alu.add
            )
            for it, (yi, yo) in enumerate(((ya, yb), (yb, ya), (ya, rn))):
                nc.vector.tensor_mul(ta[s], yi[s], yi[s])
                nc.vector.tensor_mul(tb[s], ta[s], ssq[s])
                nc.vector.tensor_scalar(ta[s], tb[s], -0.5, 1.5, Alu.mult, Alu.add)
                nc.vector.tensor_mul(yo[s], yi[s], ta[s])
            zt8 = zop.tile([P, 8, D], bf16, tag="zt8")
            rbc = (
                rn[:, t0:t0 + 2, :].rearrange("p a b -> p (a b)")
                .unsqueeze(-1).to_broadcast([P, 8, D])
            )
            nc.vector.tensor_tensor(zt8[:], pt[:], rbc, Alu.mult)
            nc.gpsimd.dma_start(
                zdram[512 * g:512 * g + 1024, :]
                .rearrange("(u p) d -> p u d", p=P),
                zt8[:],
            )
            nc.sync.dma_start_transpose(
                zT[:, g:g + 2, :].rearrange("p a b -> p (a b)"),
                zdram[512 * g:512 * g + 1024, :],
            )

        def emit_wave(block, k):
            slots = L0_SLOTS if block == "L0" else L8_SLOTS
            gset = (L0_GSETS if block == "L0" else L8_GSETS)[k]
            w = 512 * len(gset)
            for slot in slots:
                lg, lo = _chunk_cols(slot)
                lhsT = zT[:, lg, lo:lo + P]
                ps = psp.tile([P, 1536], f32, tag="ps")
                for i, g in enumerate(gset):
                    nc.tensor.matmul(
                        ps[:, 512 * i:512 * (i + 1)], lhsT, zT[:, g, :],
                        start=True, stop=True,
                    )
                # partner diagonal from raw PSUM (exact): L0 k2 has g8 at
                # tile cols [1024, 1536); partner of row 128m+p is col 128m+p
                # within that block.
                if block == "L0" and k == 2:
                    m = slot
                    sq2 = tmp.tile([P, P], f32, tag="sq2")
                    nc.vector.scalar_tensor_tensor(
                        sq2[:], ps[:, 1024 + 128 * m:1152 + 128 * m], 1.0,
                        ident[:], Alu.mult, Alu.mult,
                        accum_out=pos[:, m:m + 1],
                    )
                nc.scalar.activation(
                    es[:, slot, k, 0:w], ps[:, 0:w], Act.Exp, scale=2.0,
                    accum_out=sums[:, slot, k:k + 1],
                )
            # colsums for the groups just exp'd. Skip only this block's OWN
            # diag tile (L0@g0, L8@g8): its mirror pairs are computed
            # directly within the block. L0@g8 (macro pair (0,8), d=8) DOES
            # need a colsum - pair (8,0) is never computed directly.
            diag_g = 0 if block == "L0" else 8
            for i, g in enumerate(gset):
                if g == diag_g:
                    continue
                cs = csp.tile([64, 512], f32, tag="cs")
                for pj, s0 in enumerate((slots[0], slots[2])):
                    nc.tensor.matmul(
                        cs[:], ones2[:],
                        es[:, s0:s0 + 2, k, 512 * i:512 * (i + 1)],
                        start=(pj == 0), stop=(pj == 1), perf_mode=DR,
                    )
                sp = spp.tile([1, 512], bf16, tag="sp")
                nc.scalar.copy(sp[:], cs[0:1, :])
                nc.gpsimd.dma_start(cols_ap[CS_GROUPS.index(g), :], sp[:])

        for pi in range(8):
            emit_in(pi)
        wi = 0
        for pi in range(8):
            emit_batch_pair(pi)
            while wi < len(WAVES) and WAVES[wi][2] <= 2 * pi + 1:
                emit_wave(WAVES[wi][0], WAVES[wi][1])
                wi += 1
        assert wi == len(WAVES)

        nc.sync.dma_start(
            part_ap[:, 0:24], sums[:].rearrange("p a b -> p (a b)")
        )
        nc.sync.dma_start(part_ap[:, 24:28], pos[:])


_CACHE: dict = {}


def _compiled():
    if "nc" not in _CACHE:
        nc = bacc.Bacc(
            "TRN2", target_bir_lowering=False, debug=False,
            enable_asserts=True, num_devices=NCORES,
        )
        pc = nc.dram_tensor("pc", [N, D], bf16, kind="ExternalInput").ap()
        part = nc.dram_tensor("partial", [P, 28], f32, kind="ExternalOutput").ap()
        cols = nc.dram_tensor("cols", [15, 512], bf16, kind="ExternalOutput").ap()
        with tile.TileContext(nc) as tc:
            _build_kernel(tc, part, cols, pc)
        nc.compile()
        _CACHE["nc"] = nc
    return _CACHE["nc"]


def kernel(proj_i: np.ndarray, proj_j: np.ndarray, **run_kwargs) -> np.ndarray:
    import ml_dtypes

    assert proj_i.shape == (B, D) and proj_j.shape == (B, D)
    nc = _compiled()
    p = np.concatenate(
        [np.asarray(proj_i, np.float32), np.asarray(proj_j, np.float32)], axis=0
    ).astype(ml_dtypes.bfloat16)
    in_maps = [
        {"pc": np.ascontiguousarray(np.roll(p, -512 * c, axis=0))}
        for c in range(NCORES)
    ]
    res = run_bass_kernel_spmd(nc, in_maps, list(range(NCORES)), **run_kwargs)

    rowsum = np.zeros(N, np.float64)
    posg = np.zeros(N, np.float64)
    for c, r in enumerate(res.results):
        part = np.asarray(r["partial"], np.float64)      # [128, 28]
        cols = np.asarray(r["cols"], np.float64)         # [15, 512]
        sums = part[:, :24].reshape(P, 8, 3).sum(axis=2)  # [128, slot]
        for slot in range(8):
            base = 128 * slot if slot < 4 else 4096 + 128 * (slot - 4)
            rows = (512 * c + base + np.arange(P)) % N
            rowsum[rows] += sums[:, slot]
        for i, g in enumerate(CS_GROUPS):
            rows = (512 * c + 512 * g + np.arange(512)) % N
            rowsum[rows] += cols[i, :]
        for m in range(4):
            rows = (512 * c + 128 * m + np.arange(P)) % N
            posg[rows] = part[:, 24 + m]
            posg[(rows + B) % N] = part[:, 24 + m]
    _CACHE["last_results"] = res
    loss = -(2.0 * posg - np.log(rowsum - E2)).sum() / N
    return np.float32(loss)
